# revision 18
# baseline (speedup 1.0000x reference)
"""CrossAttentionFusion kernel for Trainium2 (8 NeuronCores, Bass/Tile).

Computation (matches the reference nn.Module):
  image_proj = relu(BN(1x1conv(image_features, image_w)))   # (B,128,H,W)
  lidar_proj = relu(BN(1x1conv(lidar_features, lidar_w)))   # (B,128,H,W)
  per (batch, 2048-pixel chunk): q = image_proj, k = v = lidar_proj
  attn_out = softmax(q k^T / sqrt(128)) @ k
  out = w0 * image_proj + w1 * attn_out,  w = softmax(modality_weights)

Sharding: the 16 independent (batch, chunk) attention problems are
distributed 2-per-core across 8 cores; each core also computes the
projections for its own pixels.  Host gathers the 8 outputs.

Per-core kernel layout notes (bf16 pipeline):
  - All matmul operands are bf16; PSUM accumulation stays fp32.  Host
    converts inputs to bf16 and folds the BN scale into the weights, so
    the projection epilogue is a single DVE tensor_scalar:
    max(psum + bias, 0) -> bf16.
  - Matmuls are 1024 wide (output spans two PSUM banks), halving the
    instruction + LDWEIGHTS count vs 512-wide halves.
  - Scores are computed k-major: ps[kslice, q]; exp() on the scalar
    engine writes bf16 (the scalar engine is the pace-setter: ~1.3us
    per 1024-wide exp, 64 of them).
  - AV uses transposed-K tiles: po[c, q] += Kpix_i^T @ ET_i, lagging
    LOOKAHEAD slices behind the score stream; the slice loop runs
    globally across both q-blocks so the PE never drains at block
    boundaries.
  - softmax denominator: binary-tree bf16 adds of the 16 ET tiles on
    the vector engine (S), then (ones/w1)^T @ S broadcast-sums across
    partitions on the PE; linv = 1/pl then already carries w1.
  - w0 is folded into the image BN affine (relu(w0*x) = w0*relu(x));
    the exp scale compensates with 1/(w0*sqrt(C)).
  - Output written per-qb as bf16; host casts back to fp32.
"""

import math
import os
import sys
from contextlib import ExitStack

import ml_dtypes
import numpy as np

sys.path.insert(0, "/opt/trn_rl_repo")

import concourse.bass as bass  # noqa: E402
import concourse.tile as tile  # noqa: E402
from concourse import bacc, bass_isa, mybir  # noqa: E402
from concourse.bass import ds, ts  # noqa: E402
from concourse.bass_utils import run_bass_kernel_spmd  # noqa: E402

F32 = mybir.dt.float32
BF16 = mybir.dt.bfloat16
NPBF16 = ml_dtypes.bfloat16

B, CL, CI, CO = 2, 256, 512, 128
H = W = 128
P = H * W                    # 16384 pixels per batch
CHUNK = 2048                 # attention chunk (pixels)
NCH = P // CHUNK             # 8 chunks per batch
NCORES = 8
UPC = (B * NCH) // NCORES    # units (b,chunk) per core = 2
EPS = 1e-5
QB = 1024                    # q-block width (one 2-bank PSUM matmul)
NQB = CHUNK // QB            # 2
KSL = CHUNK // 128           # 16 k-pixel slices per chunk
NSL = NQB * KSL              # 32 (qb, slice) score tiles per unit
NCI_IMG = CI // 128          # 4 contraction slices for image proj
NCI_LID = CL // 128          # 2 for lidar proj

_PROGRAM = None              # compiled Bass program, built once per process
LAST_RESULTS = None          # BassKernelResults of the last kernel() call


def _build_program():
    nc = bacc.Bacc("TRN2", target_bir_lowering=False, debug=False,
                   num_devices=NCORES)

    # Per-core DRAM inputs (pre-sharded, bf16 on host, constants packed so
    # the startup path is few DMA issues).
    ximg = nc.dram_tensor("ximg", [UPC, NCI_IMG, 128, CHUNK], BF16,
                          kind="ExternalInput").ap()
    xlid = nc.dram_tensor("xlid", [UPC, NCI_LID, 128, CHUNK], BF16,
                          kind="ExternalInput").ap()
    wimg = nc.dram_tensor("wimg", [128, NCI_IMG * CO], BF16,
                          kind="ExternalInput").ap()
    wlid = nc.dram_tensor("wlid", [128, NCI_LID * CO], BF16,
                          kind="ExternalInput").ap()
    # columns: img_bias, lid_bias, exp_scale, pad
    scal = nc.dram_tensor("scal", [128, 4], F32, kind="ExternalInput").ap()
    # columns 0:128 identity, 128:256 ones/w1
    idon = nc.dram_tensor("idon", [128, 256], BF16, kind="ExternalInput").ap()
    y = nc.dram_tensor("y", [UPC, NQB, CO, QB], BF16, kind="ExternalOutput").ap()

    with tile.TileContext(nc) as tc, ExitStack() as ctx:
        const = ctx.enter_context(tc.tile_pool(name="const", bufs=1))
        xi_pool = ctx.enter_context(tc.tile_pool(name="xi", bufs=UPC * NCI_IMG))
        xl_pool = ctx.enter_context(tc.tile_pool(name="xl", bufs=UPC * NCI_LID))
        proj_pool = ctx.enter_context(tc.tile_pool(name="proj", bufs=4))
        kp_pool = ctx.enter_context(tc.tile_pool(name="kp", bufs=4))
        et_pool = ctx.enter_context(tc.tile_pool(name="et", bufs=8))
        tree_pool = ctx.enter_context(tc.tile_pool(name="tree", bufs=10))
        misc_pool = ctx.enter_context(tc.tile_pool(name="misc", bufs=2))
        res_pool = ctx.enter_context(tc.tile_pool(name="res", bufs=2))
        # PSUM (8 banks of [128, 2KB]):
        #   mm 2x[128,1024]f32 (4 banks) - scores, double buffered; also
        #     borrowed for proj and transposes; the lb matmul reuses the
        #     slice-15 score tile so the rotation parity is preserved
        #   av 2x[128,1024]f32 (4 banks) - AV accumulation, double buffered
        mm_psum = ctx.enter_context(tc.tile_pool(name="mmps", bufs=2, space="PSUM"))
        av_psum = ctx.enter_context(tc.tile_pool(name="avps", bufs=2, space="PSUM"))

        # ---- startup-critical DMAs on the sync queue, in dependency order
        # for the first projection group (lidar first) ----
        # DMA issues appear to be synchronous on their issuing queue, so
        # spread the startup-critical loads over three otherwise-idle queues:
        #   sync:   lidar h0 + lidar weights + scalars  (first proj group)
        #   vector: lidar h1 + image h1                 (second proj groups)
        #   gpsimd: image weights/ident + image h0, then unit-1 inputs
        xl = {0: [xl_pool.tile([128, CHUNK], BF16, name=f"xl_0_{ci}", tag="xl")
                  for ci in range(NCI_LID)]}
        xi = {0: [xi_pool.tile([128, CHUNK], BF16, name=f"xi_0_{ci}", tag="xi")
                  for ci in range(NCI_IMG)]}
        for ci in range(NCI_LID):
            nc.sync.dma_start(xl[0][ci][:, ts(0, QB)], xlid[0, ci, :, ts(0, QB)])
        wlid_t = const.tile([128, NCI_LID * CO], BF16)
        nc.sync.dma_start(wlid_t[:], wlid)
        scal_t = const.tile([128, 4], F32)
        nc.sync.dma_start(scal_t[:], scal)
        img_b = scal_t[:, ds(0, 1)]
        lid_b = scal_t[:, ds(1, 1)]
        esc = scal_t[:, ds(2, 1)]
        w1s = scal_t[:, ds(3, 1)]
        for ci in range(NCI_LID):
            nc.scalar.dma_start(xl[0][ci][:, ts(1, QB)], xlid[0, ci, :, ts(1, QB)])
        wimg_t = const.tile([128, NCI_IMG * CO], BF16)
        nc.gpsimd.dma_start(wimg_t[:], wimg)
        idon_t = const.tile([128, 256], BF16)
        nc.gpsimd.dma_start(idon_t[:], idon)
        ident_t = idon_t[:, ds(0, 128)]
        ones_t = idon_t[:, ds(128, 128)]
        for ci in range(NCI_IMG):
            nc.gpsimd.dma_start(xi[0][ci][:, ts(0, QB)], ximg[0, ci, :, ts(0, QB)])
        for ci in range(NCI_IMG):
            nc.gpsimd.dma_start(xi[0][ci][:, ts(1, QB)], ximg[0, ci, :, ts(1, QB)])
        for u in range(1, UPC):
            xl[u] = [xl_pool.tile([128, CHUNK], BF16, name=f"xl_{u}_{ci}",
                                  tag="xl") for ci in range(NCI_LID)]
            xi[u] = [xi_pool.tile([128, CHUNK], BF16, name=f"xi_{u}_{ci}",
                                  tag="xi") for ci in range(NCI_IMG)]
            for ci in range(NCI_LID):
                nc.gpsimd.dma_start(xl[u][ci][:], xlid[u, ci])
            for ci in range(NCI_IMG):
                nc.gpsimd.dma_start(xi[u][ci][:], ximg[u, ci])

        qT = {}
        kT = {}
        kpw = {}

        def emit_proj_part1(u):
            """Lidar projection (both q-blocks), image q-block 0, and the K
            transposes.  Epilogues on the scalar engine, which is idle in
            this phase."""
            qT[u] = proj_pool.tile([128, CHUNK], BF16, name=f"qT_{u}", tag="qT")
            kT[u] = proj_pool.tile([128, CHUNK], BF16, name=f"kT_{u}", tag="kT")
            for qb in range(NQB):
                ps = mm_psum.tile([128, QB], F32, name=f"psl_{u}_{qb}", tag="ps")
                for ci in range(NCI_LID):
                    for h in range(QB // 512):
                        nc.tensor.matmul(ps[:, ts(h, 512)], wlid_t[:, ts(ci, CO)],
                                         xl[u][ci][:, ds(qb * QB + h * 512, 512)],
                                         start=(ci == 0), stop=(ci == NCI_LID - 1))
                nc.scalar.activation(kT[u][:, ts(qb, QB)], ps[:],
                                     mybir.ActivationFunctionType.Relu,
                                     bias=lid_b)
            ps = mm_psum.tile([128, QB], F32, name=f"psi_{u}_0", tag="ps")
            for ci in range(NCI_IMG):
                for h in range(QB // 512):
                    nc.tensor.matmul(ps[:, ts(h, 512)], wimg_t[:, ts(ci, CO)],
                                     xi[u][ci][:, ds(h * 512, 512)],
                                     start=(ci == 0), stop=(ci == NCI_IMG - 1))
            nc.scalar.activation(qT[u][:, ds(0, QB)], ps[:],
                                 mybir.ActivationFunctionType.Relu,
                                 bias=img_b)
            kpw[u] = []

        def emit_transpose_group(u, g):
            """One 8-slice K transpose group, PSUM borrowed from the (still
            idle) AV pool so the score-tile rotation is untouched; rides the
            early-stream PE slack before the AV matmuls begin."""
            pt = av_psum.tile([128, 8 * 128], BF16, name=f"pt_{u}_{g}",
                              tag="av")
            for k in range(8):
                nc.tensor.transpose(pt[:, ts(k, 128)],
                                    kT[u][:, ts(g * 8 + k, 128)], ident_t)
            kpt = kp_pool.tile([128, 8 * 128], BF16, name=f"kp_{u}_{g}",
                               tag="kp")
            nc.vector.tensor_copy(kpt[:], pt[:])
            kpw[u].append(kpt)

        def emit_proj_part2(u):
            """Image q-block 1, slotted into the attention stream (PSUM from
            the AV pool, relu on the DVE) so neither the score-tile rotation
            nor the scalar engine's exp stream is interrupted."""
            ps = av_psum.tile([128, QB], F32, name=f"psi_{u}_1", tag="av")
            for ci in range(NCI_IMG):
                for h in range(QB // 512):
                    nc.tensor.matmul(ps[:, ts(h, 512)], wimg_t[:, ts(ci, CO)],
                                     xi[u][ci][:, ds(QB + h * 512, 512)],
                                     start=(ci == 0), stop=(ci == NCI_IMG - 1))
            nc.vector.tensor_scalar(qT[u][:, ds(QB, QB)], ps[:], img_b, 0.0,
                                    op0=mybir.AluOpType.add,
                                    op1=mybir.AluOpType.max)

        emit_proj_part1(0)
        for u in range(UPC):
            # ---- attention: one global slice pipeline across both q-blocks.
            # scores/exp run LOOKAHEAD slices ahead of the AV matmuls so the
            # in-order PE queue never waits on the scalar engine. ----
            LOOKAHEAD = 3
            ets = [None] * NSL
            pos = [None] * NQB
            ps_sc = [None] * NSL
            lvl1 = [None] * (KSL // 2)
            lvl2 = [None] * (KSL // 4)
            lvl3 = {}

            def qb_epilogue(qb, l3):
                """Denominator broadcast ((ones/w1)^T @ lvl3 parts) into an
                AV-pool PSUM tile -- the score-tile rotation never sees it --
                then reciprocal + blend + store in 512-wide halves so the
                chain pipelines on the DVE."""
                pl = av_psum.tile([128, QB], F32, name=f"pl_{u}_{qb}",
                                  tag="av")
                for h in range(QB // 512):
                    for part in range(2):
                        nc.tensor.matmul(pl[:, ts(h, 512)], ones_t,
                                         l3[part][:, ts(h, 512)],
                                         start=(part == 0), stop=(part == 1))
                linv = misc_pool.tile([128, QB], F32, name=f"linv_{u}_{qb}",
                                      tag="linv")
                tmp = misc_pool.tile([128, QB], F32, name=f"tmp_{u}_{qb}",
                                     tag="tmp")
                res = res_pool.tile([128, QB], BF16, name=f"res_{u}_{qb}",
                                    tag="res")
                for h in range(QB // 512):
                    nc.vector.reciprocal_approx_fast(linv[:, ts(h, 512)],
                                                     pl[:, ts(h, 512)])
                    nc.vector.tensor_mul(tmp[:, ts(h, 512)],
                                         pos[qb][:, ts(h, 512)],
                                         linv[:, ts(h, 512)])
                    nc.vector.tensor_add(res[:, ts(h, 512)], tmp[:, ts(h, 512)],
                                         qT[u][:, ds(qb * QB + h * 512, 512)])
                    nc.sync.dma_start(y[u, qb, :, ts(h, 512)],
                                      res[:, ts(h, 512)])

            for g in range(NSL + LOOKAHEAD):
                if g < NSL:
                    qb, i = divmod(g, KSL)
                    ps = mm_psum.tile([128, QB], F32,
                                      name=f"pss_{u}_{qb}_{i}", tag="ps")
                    ps_sc[g] = ps
                    for h in range(QB // 512):
                        nc.tensor.matmul(ps[:, ts(h, 512)], kT[u][:, ts(i, 128)],
                                         qT[u][:, ds(qb * QB + h * 512, 512)],
                                         start=True, stop=True)
                    et = et_pool.tile([128, QB], BF16,
                                      name=f"et_{u}_{qb}_{i}", tag="et")
                    nc.scalar.activation(et[:], ps[:],
                                         mybir.ActivationFunctionType.Exp,
                                         scale=esc)
                    ets[g] = et
                    # binary-tree accumulation of S (all-bf16 2x DVE adds)
                    if i % 2 == 1:
                        t = tree_pool.tile([128, QB], BF16,
                                           name=f"t1_{u}_{qb}_{i}", tag="t1")
                        nc.vector.tensor_add(t[:], ets[g - 1][:], et[:])
                        lvl1[i // 2] = t
                        if i % 4 == 3:
                            t2 = tree_pool.tile([128, QB], BF16,
                                                name=f"t2_{u}_{qb}_{i}",
                                                tag="t1")
                            nc.vector.tensor_add(t2[:], lvl1[i // 2 - 1][:],
                                                 t[:])
                            lvl2[i // 4] = t2
                            if i % 8 == 7:
                                t3 = tree_pool.tile([128, QB], BF16,
                                                    name=f"t3_{u}_{qb}_{i}",
                                                    tag="t1")
                                nc.vector.tensor_add(t3[:],
                                                     lvl2[i // 4 - 1][:],
                                                     t2[:])
                                lvl3[(qb, i // 8)] = t3
                    if 1 <= g <= 2:
                        emit_transpose_group(u, g - 1)
                    elif g == 3:
                        emit_proj_part2(u)
                elif g == NSL and u + 1 < UPC:
                    # overlap the next unit's projections with this unit's
                    # AV drain + epilogue (scalar engine idles here anyway)
                    emit_proj_part1(u + 1)
                j = g - LOOKAHEAD
                if j >= 0:
                    qbj, i = divmod(j, KSL)
                    if i == 0:
                        pos[qbj] = av_psum.tile([128, QB], F32,
                                                name=f"po_{u}_{qbj}", tag="av")
                    kslice = kpw[u][i // 8][:, ts(i % 8, 128)]
                    for h in range(QB // 512):
                        nc.tensor.matmul(pos[qbj][:, ts(h, 512)], kslice,
                                         ets[j][:, ts(h, 512)],
                                         start=(i == 0), stop=(i == KSL - 1))
                    if i == KSL - 1:
                        qb_epilogue(qbj, [lvl3[(qbj, 0)], lvl3[(qbj, 1)]])

    nc.compile()
    return nc


def _shard_inputs(inputs):
    """Build the 8 per-core input maps from the full input dict."""
    mw = np.asarray(inputs["modality_weights"], np.float64)
    e = np.exp(mw - mw.max())
    w = (e / e.sum()).astype(np.float64)
    w0, w1 = float(w[0]), float(w[1])

    def bn_fold(gamma, beta, mean, var, mul):
        g = np.asarray(gamma, np.float64)
        b = np.asarray(beta, np.float64)
        m = np.asarray(mean, np.float64)
        v = np.asarray(var, np.float64)
        scale = g / np.sqrt(v + EPS) * mul
        bias = (b - m * g / np.sqrt(v + EPS)) * mul
        return scale, bias.astype(np.float32)

    i_s, i_b = bn_fold(inputs["image_gamma"], inputs["image_beta"],
                       inputs["image_mean"], inputs["image_var"], w0)
    l_s, l_b = bn_fold(inputs["lidar_gamma"], inputs["lidar_beta"],
                       inputs["lidar_mean"], inputs["lidar_var"], 1.0)

    # weight slices with the BN scale folded in, packed [cin(128), ci*CO]
    wi = (np.asarray(inputs["image_w"], np.float64) * i_s[:, None]).T.reshape(
        NCI_IMG, 128, CO).transpose(1, 0, 2).reshape(128, NCI_IMG * CO)
    wi = np.ascontiguousarray(wi).astype(NPBF16)
    wl = (np.asarray(inputs["lidar_w"], np.float64) * l_s[:, None]).T.reshape(
        NCI_LID, 128, CO).transpose(1, 0, 2).reshape(128, NCI_LID * CO)
    wl = np.ascontiguousarray(wl).astype(NPBF16)

    scal = np.zeros((128, 4), np.float32)
    scal[:, 0] = i_b
    scal[:, 1] = l_b
    scal[:, 2] = 1.0 / (w0 * math.sqrt(CO))
    scal[:, 3] = w1
    idon = np.zeros((128, 256), np.float32)
    idon[:, :128] = np.eye(128, dtype=np.float32)
    # ones carry 1/w1 so linv = 1/pl = w1/denominator
    idon[:, 128:] = 1.0 / w1
    idon = idon.astype(NPBF16)

    # full features reshaped to (B, nchunks, C, 2048), bf16
    img = np.asarray(inputs["image_features"], np.float32).reshape(
        B, CI, NCH, CHUNK).astype(NPBF16)
    lid = np.asarray(inputs["lidar_features"], np.float32).reshape(
        B, CL, NCH, CHUNK).astype(NPBF16)

    in_maps = []
    for core in range(NCORES):
        ximg = np.empty((UPC, NCI_IMG, 128, CHUNK), NPBF16)
        xlid = np.empty((UPC, NCI_LID, 128, CHUNK), NPBF16)
        for ul in range(UPC):
            un = core * UPC + ul
            b, c = un // NCH, un % NCH
            ximg[ul] = img[b, :, c, :].reshape(NCI_IMG, 128, CHUNK)
            xlid[ul] = lid[b, :, c, :].reshape(NCI_LID, 128, CHUNK)
        in_maps.append({
            "ximg": ximg, "xlid": xlid, "wimg": wi, "wlid": wl,
            "scal": scal, "idon": idon,
        })
    return in_maps


def kernel(**inputs) -> np.ndarray:
    global _PROGRAM, LAST_RESULTS
    if _PROGRAM is None:
        _PROGRAM = _build_program()
    nc = _PROGRAM

    in_maps = _shard_inputs(inputs)
    trace = os.environ.get("BASS_KERNEL_TRACE", "0") == "1"
    tmpdir = os.environ.get("BASS_KERNEL_TRACE_DIR") or None
    if tmpdir:
        os.makedirs(tmpdir, exist_ok=True)
    results = run_bass_kernel_spmd(nc, in_maps, core_ids=list(range(NCORES)),
                                   trace=trace, tmpdir=tmpdir)
    LAST_RESULTS = results

    out = np.empty((B, CO, H, W), np.float32)
    outv = out.reshape(B, CO, NCH, NQB, QB)
    for core in range(NCORES):
        yc = np.asarray(results.results[core]["y"], dtype=np.float32)
        for ul in range(UPC):
            un = core * UPC + ul
            b, c = un // NCH, un % NCH
            outv[b, :, c, :, :] = yc[ul].transpose(1, 0, 2)
    return out


if __name__ == "__main__":
    rng = np.random.default_rng(0)
    inputs = {
        "lidar_features": rng.standard_normal((B, CL, H, W), np.float32),
        "image_features": rng.standard_normal((B, CI, H, W), np.float32),
        "lidar_w": rng.standard_normal((CO, CL), np.float32) * np.sqrt(2.0 / CO),
        "lidar_gamma": np.ones(CO, np.float32),
        "lidar_beta": np.zeros(CO, np.float32),
        "lidar_mean": rng.standard_normal(CO).astype(np.float32) * 0.1,
        "lidar_var": rng.uniform(0.5, 1.5, CO).astype(np.float32),
        "image_w": rng.standard_normal((CO, CI), np.float32) * np.sqrt(2.0 / CO),
        "image_gamma": np.ones(CO, np.float32),
        "image_beta": np.zeros(CO, np.float32),
        "image_mean": rng.standard_normal(CO).astype(np.float32) * 0.1,
        "image_var": rng.uniform(0.5, 1.5, CO).astype(np.float32),
        "modality_weights": np.ones(2, np.float32),
    }
    out = kernel(**inputs)
    print("kernel out:", out.shape, out.dtype, float(np.abs(out).mean()))


# revision 19
# speedup vs baseline: 1.1490x; 1.1490x over previous
"""CrossAttentionFusion kernel for Trainium2 (8 NeuronCores, Bass/Tile).

Computation (matches the reference nn.Module):
  image_proj = relu(BN(1x1conv(image_features, image_w)))   # (B,128,H,W)
  lidar_proj = relu(BN(1x1conv(lidar_features, lidar_w)))   # (B,128,H,W)
  per (batch, 2048-pixel chunk): q = image_proj, k = v = lidar_proj
  attn_out = softmax(q k^T / sqrt(128)) @ k
  out = w0 * image_proj + w1 * attn_out,  w = softmax(modality_weights)

Sharding: the 16 independent (batch, chunk) attention problems are
distributed 2-per-core across 8 cores; each core also computes the
projections for its own pixels.  Host gathers the 8 outputs.

Per-core kernel layout notes (bf16 pipeline):
  - All matmul operands are bf16; PSUM accumulation stays fp32.  Host
    converts inputs to bf16 and folds the BN scale into the weights, so
    the projection epilogue is a single DVE tensor_scalar:
    max(psum + bias, 0) -> bf16.
  - Matmuls are 1024 wide (output spans two PSUM banks), halving the
    instruction + LDWEIGHTS count vs 512-wide halves.
  - Scores are computed k-major: ps[kslice, q]; exp() on the scalar
    engine writes bf16 (the scalar engine is the pace-setter: ~1.3us
    per 1024-wide exp, 64 of them).
  - AV uses transposed-K tiles: po[c, q] += Kpix_i^T @ ET_i, lagging
    LOOKAHEAD slices behind the score stream; the slice loop runs
    globally across both q-blocks so the PE never drains at block
    boundaries.
  - softmax denominator: binary-tree bf16 adds of the 16 ET tiles on
    the vector engine (S), then (ones/w1)^T @ S broadcast-sums across
    partitions on the PE; linv = 1/pl then already carries w1.
  - w0 is folded into the image BN affine (relu(w0*x) = w0*relu(x));
    the exp scale compensates with 1/(w0*sqrt(C)).
  - Output written per-qb as bf16; host casts back to fp32.
"""

import math
import os
import sys
from contextlib import ExitStack

import ml_dtypes
import numpy as np

sys.path.insert(0, "/opt/trn_rl_repo")

import concourse.bass as bass  # noqa: E402
import concourse.tile as tile  # noqa: E402
from concourse import bacc, bass_isa, mybir  # noqa: E402
from concourse.bass import ds, ts  # noqa: E402
from concourse.bass_utils import run_bass_kernel_spmd  # noqa: E402

F32 = mybir.dt.float32
BF16 = mybir.dt.bfloat16
NPBF16 = ml_dtypes.bfloat16

B, CL, CI, CO = 2, 256, 512, 128
H = W = 128
P = H * W                    # 16384 pixels per batch
CHUNK = 2048                 # attention chunk (pixels)
NCH = P // CHUNK             # 8 chunks per batch
NCORES = 8
UPC = (B * NCH) // NCORES    # units (b,chunk) per core = 2
EPS = 1e-5
QB = 1024                    # q-block width (one 2-bank PSUM matmul)
NQB = CHUNK // QB            # 2
KSL = CHUNK // 128           # 16 k-pixel slices per chunk
NSL = NQB * KSL              # 32 (qb, slice) score tiles per unit
NCI_IMG = CI // 128          # 4 contraction slices for image proj
NCI_LID = CL // 128          # 2 for lidar proj

_PROGRAM = None              # compiled Bass program, built once per process
LAST_RESULTS = None          # BassKernelResults of the last kernel() call


def _build_program():
    nc = bacc.Bacc("TRN2", target_bir_lowering=False, debug=False,
                   num_devices=NCORES)

    # Per-core DRAM inputs (pre-sharded, bf16 on host, constants packed so
    # the startup path is few DMA issues).
    ximg = nc.dram_tensor("ximg", [UPC, NCI_IMG, 128, CHUNK], BF16,
                          kind="ExternalInput").ap()
    xlid = nc.dram_tensor("xlid", [UPC, NCI_LID, 128, CHUNK], BF16,
                          kind="ExternalInput").ap()
    wimg = nc.dram_tensor("wimg", [128, NCI_IMG * CO], BF16,
                          kind="ExternalInput").ap()
    wlid = nc.dram_tensor("wlid", [128, NCI_LID * CO], BF16,
                          kind="ExternalInput").ap()
    # columns: img_bias, lid_bias, exp_scale, pad
    scal = nc.dram_tensor("scal", [128, 4], F32, kind="ExternalInput").ap()
    # columns 0:128 identity, 128:256 ones/w1
    idon = nc.dram_tensor("idon", [128, 256], BF16, kind="ExternalInput").ap()
    y = nc.dram_tensor("y", [UPC, NQB, CO, QB], BF16, kind="ExternalOutput").ap()

    with tile.TileContext(nc) as tc, ExitStack() as ctx:
        const = ctx.enter_context(tc.tile_pool(name="const", bufs=1))
        xi_pool = ctx.enter_context(tc.tile_pool(name="xi", bufs=UPC * NCI_IMG))
        xl_pool = ctx.enter_context(tc.tile_pool(name="xl", bufs=UPC * NCI_LID))
        proj_pool = ctx.enter_context(tc.tile_pool(name="proj", bufs=4))
        kp_pool = ctx.enter_context(tc.tile_pool(name="kp", bufs=4))
        et_pool = ctx.enter_context(tc.tile_pool(name="et", bufs=8))
        tree_pool = ctx.enter_context(tc.tile_pool(name="tree", bufs=10))
        misc_pool = ctx.enter_context(tc.tile_pool(name="misc", bufs=2))
        res_pool = ctx.enter_context(tc.tile_pool(name="res", bufs=2))
        # PSUM (8 banks of [128, 2KB]):
        #   mm 2x[128,1024]f32 (4 banks) - scores, double buffered; also
        #     borrowed for proj and transposes; the lb matmul reuses the
        #     slice-15 score tile so the rotation parity is preserved
        #   av 2x[128,1024]f32 (4 banks) - AV accumulation, double buffered
        mm_psum = ctx.enter_context(tc.tile_pool(name="mmps", bufs=2, space="PSUM"))
        av_psum = ctx.enter_context(tc.tile_pool(name="avps", bufs=2, space="PSUM"))

        # ---- startup-critical DMAs on the sync queue, in dependency order
        # for the first projection group (lidar first) ----
        # DMA issues appear to be synchronous on their issuing queue, so
        # spread the startup-critical loads over three otherwise-idle queues:
        #   sync:   lidar h0 + lidar weights + scalars  (first proj group)
        #   vector: lidar h1 + image h1                 (second proj groups)
        #   gpsimd: image weights/ident + image h0, then unit-1 inputs
        xl = {0: [xl_pool.tile([128, CHUNK], BF16, name=f"xl_0_{ci}", tag="xl")
                  for ci in range(NCI_LID)]}
        xi = {0: [xi_pool.tile([128, CHUNK], BF16, name=f"xi_0_{ci}", tag="xi")
                  for ci in range(NCI_IMG)]}
        for ci in range(NCI_LID):
            nc.sync.dma_start(xl[0][ci][:, ts(0, QB)], xlid[0, ci, :, ts(0, QB)])
        wlid_t = const.tile([128, NCI_LID * CO], BF16)
        nc.sync.dma_start(wlid_t[:], wlid)
        scal_t = const.tile([128, 4], F32)
        nc.sync.dma_start(scal_t[:], scal)
        img_b = scal_t[:, ds(0, 1)]
        lid_b = scal_t[:, ds(1, 1)]
        esc = scal_t[:, ds(2, 1)]
        w1s = scal_t[:, ds(3, 1)]
        for ci in range(NCI_LID):
            nc.scalar.dma_start(xl[0][ci][:, ts(1, QB)], xlid[0, ci, :, ts(1, QB)])
        wimg_t = const.tile([128, NCI_IMG * CO], BF16)
        nc.gpsimd.dma_start(wimg_t[:], wimg)
        idon_t = const.tile([128, 256], BF16)
        nc.gpsimd.dma_start(idon_t[:], idon)
        ident_t = idon_t[:, ds(0, 128)]
        ones_t = idon_t[:, ds(128, 128)]
        for ci in range(NCI_IMG):
            nc.gpsimd.dma_start(xi[0][ci][:, ts(0, QB)], ximg[0, ci, :, ts(0, QB)])
        for ci in range(NCI_IMG):
            nc.gpsimd.dma_start(xi[0][ci][:, ts(1, QB)], ximg[0, ci, :, ts(1, QB)])
        for u in range(1, UPC):
            xl[u] = [xl_pool.tile([128, CHUNK], BF16, name=f"xl_{u}_{ci}",
                                  tag="xl") for ci in range(NCI_LID)]
            xi[u] = [xi_pool.tile([128, CHUNK], BF16, name=f"xi_{u}_{ci}",
                                  tag="xi") for ci in range(NCI_IMG)]
            for ci in range(NCI_LID):
                nc.gpsimd.dma_start(xl[u][ci][:], xlid[u, ci])
            for ci in range(NCI_IMG):
                nc.gpsimd.dma_start(xi[u][ci][:], ximg[u, ci])

        qT = {}
        kT = {}
        kpw = {}

        def emit_proj_part1(u):
            """Lidar projection (both q-blocks), image q-block 0, and the K
            transposes.  Epilogues on the scalar engine, which is idle in
            this phase."""
            qT[u] = proj_pool.tile([128, CHUNK], BF16, name=f"qT_{u}", tag="qT")
            kT[u] = proj_pool.tile([128, CHUNK], BF16, name=f"kT_{u}", tag="kT")
            for qb in range(NQB):
                ps = mm_psum.tile([128, QB], F32, name=f"psl_{u}_{qb}", tag="ps")
                for ci in range(NCI_LID):
                    for h in range(QB // 512):
                        nc.tensor.matmul(ps[:, ts(h, 512)], wlid_t[:, ts(ci, CO)],
                                         xl[u][ci][:, ds(qb * QB + h * 512, 512)],
                                         start=(ci == 0), stop=(ci == NCI_LID - 1))
                nc.scalar.activation(kT[u][:, ts(qb, QB)], ps[:],
                                     mybir.ActivationFunctionType.Relu,
                                     bias=lid_b)
            ps = mm_psum.tile([128, QB], F32, name=f"psi_{u}_0", tag="ps")
            for ci in range(NCI_IMG):
                for h in range(QB // 512):
                    nc.tensor.matmul(ps[:, ts(h, 512)], wimg_t[:, ts(ci, CO)],
                                     xi[u][ci][:, ds(h * 512, 512)],
                                     start=(ci == 0), stop=(ci == NCI_IMG - 1))
            nc.scalar.activation(qT[u][:, ds(0, QB)], ps[:],
                                 mybir.ActivationFunctionType.Relu,
                                 bias=img_b)
            kpw[u] = []

        def emit_transpose_group(u, g):
            """One 8-slice K transpose group, PSUM borrowed from the (still
            idle) AV pool so the score-tile rotation is untouched; rides the
            early-stream PE slack before the AV matmuls begin."""
            pt = av_psum.tile([128, 8 * 128], BF16, name=f"pt_{u}_{g}",
                              tag="av")
            for k in range(8):
                nc.tensor.transpose(pt[:, ts(k, 128)],
                                    kT[u][:, ts(g * 8 + k, 128)], ident_t)
            kpt = kp_pool.tile([128, 8 * 128], BF16, name=f"kp_{u}_{g}",
                               tag="kp")
            nc.vector.tensor_copy(kpt[:], pt[:])
            kpw[u].append(kpt)

        def emit_proj_part2(u):
            """Image q-block 1, slotted into the attention stream (PSUM from
            the AV pool, relu on the DVE) so neither the score-tile rotation
            nor the scalar engine's exp stream is interrupted."""
            ps = av_psum.tile([128, QB], F32, name=f"psi_{u}_1", tag="av")
            for ci in range(NCI_IMG):
                for h in range(QB // 512):
                    nc.tensor.matmul(ps[:, ts(h, 512)], wimg_t[:, ts(ci, CO)],
                                     xi[u][ci][:, ds(QB + h * 512, 512)],
                                     start=(ci == 0), stop=(ci == NCI_IMG - 1))
            nc.vector.tensor_scalar(qT[u][:, ds(QB, QB)], ps[:], img_b, 0.0,
                                    op0=mybir.AluOpType.add,
                                    op1=mybir.AluOpType.max)

        emit_proj_part1(0)
        for u in range(UPC):
            # ---- attention: one global slice pipeline across both q-blocks.
            # scores/exp run LOOKAHEAD slices ahead of the AV matmuls so the
            # in-order PE queue never waits on the scalar engine. ----
            LOOKAHEAD = 3
            ets = [None] * NSL
            pos = [None] * NQB
            ps_sc = [None] * NSL
            lvl1 = [None] * (KSL // 2)
            lvl2 = [None] * (KSL // 4)
            lvl3 = {}

            def qb_epilogue(qb, parts, pl):
                """Denominator broadcast ((ones/w1)^T @ partial S tiles, a
                4-way PSUM accumulation so only one cheap DVE add separates
                the last exp from the lb matmul) into the consumed slice-15
                score tile, then reciprocal + blend + store in 512-wide
                halves so the chain pipelines on the DVE."""
                for h in range(QB // 512):
                    for pi, part in enumerate(parts):
                        nc.tensor.matmul(pl[:, ts(h, 512)], ones_t,
                                         part[:, ts(h, 512)],
                                         start=(pi == 0),
                                         stop=(pi == len(parts) - 1))
                linv = misc_pool.tile([128, QB], F32, name=f"linv_{u}_{qb}",
                                      tag="linv")
                tmp = misc_pool.tile([128, QB], F32, name=f"tmp_{u}_{qb}",
                                     tag="tmp")
                res = res_pool.tile([128, QB], BF16, name=f"res_{u}_{qb}",
                                    tag="res")
                for h in range(QB // 512):
                    nc.vector.reciprocal_approx_fast(linv[:, ts(h, 512)],
                                                     pl[:, ts(h, 512)])
                    nc.vector.tensor_mul(tmp[:, ts(h, 512)],
                                         pos[qb][:, ts(h, 512)],
                                         linv[:, ts(h, 512)])
                    nc.vector.tensor_add(res[:, ts(h, 512)], tmp[:, ts(h, 512)],
                                         qT[u][:, ds(qb * QB + h * 512, 512)])
                    nc.sync.dma_start(y[u, qb, :, ts(h, 512)],
                                      res[:, ts(h, 512)])

            for g in range(NSL + LOOKAHEAD):
                if g < NSL:
                    qb, i = divmod(g, KSL)
                    ps = mm_psum.tile([128, QB], F32,
                                      name=f"pss_{u}_{qb}_{i}", tag="ps")
                    ps_sc[g] = ps
                    for h in range(QB // 512):
                        nc.tensor.matmul(ps[:, ts(h, 512)], kT[u][:, ts(i, 128)],
                                         qT[u][:, ds(qb * QB + h * 512, 512)],
                                         start=True, stop=True)
                    et = et_pool.tile([128, QB], BF16,
                                      name=f"et_{u}_{qb}_{i}", tag="et")
                    nc.scalar.activation(et[:], ps[:],
                                         mybir.ActivationFunctionType.Exp,
                                         scale=esc)
                    ets[g] = et
                    # binary-tree accumulation of S (all-bf16 2x DVE adds)
                    if i % 2 == 1:
                        t = tree_pool.tile([128, QB], BF16,
                                           name=f"t1_{u}_{qb}_{i}", tag="t1")
                        nc.vector.tensor_add(t[:], ets[g - 1][:], et[:])
                        lvl1[i // 2] = t
                        if i % 4 == 3 and i < 12:
                            t2 = tree_pool.tile([128, QB], BF16,
                                                name=f"t2_{u}_{qb}_{i}",
                                                tag="t1")
                            nc.vector.tensor_add(t2[:], lvl1[i // 2 - 1][:],
                                                 t[:])
                            lvl2[i // 4] = t2
                            if i == 7:
                                t3 = tree_pool.tile([128, QB], BF16,
                                                    name=f"t3_{u}_{qb}_{i}",
                                                    tag="t1")
                                nc.vector.tensor_add(t3[:],
                                                     lvl2[0][:], t2[:])
                                lvl3[(qb, 0)] = t3
                    if 1 <= g <= 2:
                        emit_transpose_group(u, g - 1)
                    elif g == 3:
                        emit_proj_part2(u)
                elif g == NSL and u + 1 < UPC:
                    # overlap the next unit's projections with this unit's
                    # AV drain + epilogue (scalar engine idles here anyway)
                    emit_proj_part1(u + 1)
                j = g - LOOKAHEAD
                if j >= 0:
                    qbj, i = divmod(j, KSL)
                    if i == 0:
                        pos[qbj] = av_psum.tile([128, QB], F32,
                                                name=f"po_{u}_{qbj}", tag="av")
                    kslice = kpw[u][i // 8][:, ts(i % 8, 128)]
                    for h in range(QB // 512):
                        nc.tensor.matmul(pos[qbj][:, ts(h, 512)], kslice,
                                         ets[j][:, ts(h, 512)],
                                         start=(i == 0), stop=(i == KSL - 1))
                    if i == KSL - 1:
                        qb_epilogue(qbj,
                                    [lvl3[(qbj, 0)], lvl2[2], lvl1[6],
                                     lvl1[7]], ps_sc[j])

    nc.compile()
    return nc


def _shard_inputs(inputs):
    """Build the 8 per-core input maps from the full input dict."""
    mw = np.asarray(inputs["modality_weights"], np.float64)
    e = np.exp(mw - mw.max())
    w = (e / e.sum()).astype(np.float64)
    w0, w1 = float(w[0]), float(w[1])

    def bn_fold(gamma, beta, mean, var, mul):
        g = np.asarray(gamma, np.float64)
        b = np.asarray(beta, np.float64)
        m = np.asarray(mean, np.float64)
        v = np.asarray(var, np.float64)
        scale = g / np.sqrt(v + EPS) * mul
        bias = (b - m * g / np.sqrt(v + EPS)) * mul
        return scale, bias.astype(np.float32)

    i_s, i_b = bn_fold(inputs["image_gamma"], inputs["image_beta"],
                       inputs["image_mean"], inputs["image_var"], w0)
    l_s, l_b = bn_fold(inputs["lidar_gamma"], inputs["lidar_beta"],
                       inputs["lidar_mean"], inputs["lidar_var"], 1.0)

    # weight slices with the BN scale folded in, packed [cin(128), ci*CO]
    wi = (np.asarray(inputs["image_w"], np.float64) * i_s[:, None]).T.reshape(
        NCI_IMG, 128, CO).transpose(1, 0, 2).reshape(128, NCI_IMG * CO)
    wi = np.ascontiguousarray(wi).astype(NPBF16)
    wl = (np.asarray(inputs["lidar_w"], np.float64) * l_s[:, None]).T.reshape(
        NCI_LID, 128, CO).transpose(1, 0, 2).reshape(128, NCI_LID * CO)
    wl = np.ascontiguousarray(wl).astype(NPBF16)

    scal = np.zeros((128, 4), np.float32)
    scal[:, 0] = i_b
    scal[:, 1] = l_b
    scal[:, 2] = 1.0 / (w0 * math.sqrt(CO))
    scal[:, 3] = w1
    idon = np.zeros((128, 256), np.float32)
    idon[:, :128] = np.eye(128, dtype=np.float32)
    # ones carry 1/w1 so linv = 1/pl = w1/denominator
    idon[:, 128:] = 1.0 / w1
    idon = idon.astype(NPBF16)

    # full features reshaped to (B, nchunks, C, 2048), bf16
    img = np.asarray(inputs["image_features"], np.float32).reshape(
        B, CI, NCH, CHUNK).astype(NPBF16)
    lid = np.asarray(inputs["lidar_features"], np.float32).reshape(
        B, CL, NCH, CHUNK).astype(NPBF16)

    in_maps = []
    for core in range(NCORES):
        ximg = np.empty((UPC, NCI_IMG, 128, CHUNK), NPBF16)
        xlid = np.empty((UPC, NCI_LID, 128, CHUNK), NPBF16)
        for ul in range(UPC):
            un = core * UPC + ul
            b, c = un // NCH, un % NCH
            ximg[ul] = img[b, :, c, :].reshape(NCI_IMG, 128, CHUNK)
            xlid[ul] = lid[b, :, c, :].reshape(NCI_LID, 128, CHUNK)
        in_maps.append({
            "ximg": ximg, "xlid": xlid, "wimg": wi, "wlid": wl,
            "scal": scal, "idon": idon,
        })
    return in_maps


def kernel(**inputs) -> np.ndarray:
    global _PROGRAM, LAST_RESULTS
    if _PROGRAM is None:
        _PROGRAM = _build_program()
    nc = _PROGRAM

    in_maps = _shard_inputs(inputs)
    trace = os.environ.get("BASS_KERNEL_TRACE", "0") == "1"
    tmpdir = os.environ.get("BASS_KERNEL_TRACE_DIR") or None
    if tmpdir:
        os.makedirs(tmpdir, exist_ok=True)
    results = run_bass_kernel_spmd(nc, in_maps, core_ids=list(range(NCORES)),
                                   trace=trace, tmpdir=tmpdir)
    LAST_RESULTS = results

    out = np.empty((B, CO, H, W), np.float32)
    outv = out.reshape(B, CO, NCH, NQB, QB)
    for core in range(NCORES):
        yc = np.asarray(results.results[core]["y"], dtype=np.float32)
        for ul in range(UPC):
            un = core * UPC + ul
            b, c = un // NCH, un % NCH
            outv[b, :, c, :, :] = yc[ul].transpose(1, 0, 2)
    return out


if __name__ == "__main__":
    rng = np.random.default_rng(0)
    inputs = {
        "lidar_features": rng.standard_normal((B, CL, H, W), np.float32),
        "image_features": rng.standard_normal((B, CI, H, W), np.float32),
        "lidar_w": rng.standard_normal((CO, CL), np.float32) * np.sqrt(2.0 / CO),
        "lidar_gamma": np.ones(CO, np.float32),
        "lidar_beta": np.zeros(CO, np.float32),
        "lidar_mean": rng.standard_normal(CO).astype(np.float32) * 0.1,
        "lidar_var": rng.uniform(0.5, 1.5, CO).astype(np.float32),
        "image_w": rng.standard_normal((CO, CI), np.float32) * np.sqrt(2.0 / CO),
        "image_gamma": np.ones(CO, np.float32),
        "image_beta": np.zeros(CO, np.float32),
        "image_mean": rng.standard_normal(CO).astype(np.float32) * 0.1,
        "image_var": rng.uniform(0.5, 1.5, CO).astype(np.float32),
        "modality_weights": np.ones(2, np.float32),
    }
    out = kernel(**inputs)
    print("kernel out:", out.shape, out.dtype, float(np.abs(out).mean()))


# revision 20
# speedup vs baseline: 1.2026x; 1.0467x over previous
"""CrossAttentionFusion kernel for Trainium2 (8 NeuronCores, Bass/Tile).

Computation (matches the reference nn.Module):
  image_proj = relu(BN(1x1conv(image_features, image_w)))   # (B,128,H,W)
  lidar_proj = relu(BN(1x1conv(lidar_features, lidar_w)))   # (B,128,H,W)
  per (batch, 2048-pixel chunk): q = image_proj, k = v = lidar_proj
  attn_out = softmax(q k^T / sqrt(128)) @ k
  out = w0 * image_proj + w1 * attn_out,  w = softmax(modality_weights)

Sharding: the 16 independent (batch, chunk) attention problems are
distributed 2-per-core across 8 cores; each core also computes the
projections for its own pixels.  Host gathers the 8 outputs.

Per-core kernel layout notes (bf16 pipeline):
  - All matmul operands are bf16; PSUM accumulation stays fp32.  Host
    converts inputs to bf16 and folds the BN scale into the weights, so
    the projection epilogue is a single DVE tensor_scalar:
    max(psum + bias, 0) -> bf16.
  - Matmuls are 1024 wide (output spans two PSUM banks), halving the
    instruction + LDWEIGHTS count vs 512-wide halves.
  - Scores are computed k-major: ps[kslice, q]; exp() on the scalar
    engine writes bf16 (the scalar engine is the pace-setter: ~1.3us
    per 1024-wide exp, 64 of them).
  - AV uses transposed-K tiles: po[c, q] += Kpix_i^T @ ET_i, lagging
    LOOKAHEAD slices behind the score stream; the slice loop runs
    globally across both q-blocks so the PE never drains at block
    boundaries.
  - softmax denominator: binary-tree bf16 adds of the 16 ET tiles on
    the vector engine (S), then (ones/w1)^T @ S broadcast-sums across
    partitions on the PE; linv = 1/pl then already carries w1.
  - w0 is folded into the image BN affine (relu(w0*x) = w0*relu(x));
    the exp scale compensates with 1/(w0*sqrt(C)).
  - Output written per-qb as bf16; host casts back to fp32.
"""

import math
import os
import sys
from contextlib import ExitStack

import ml_dtypes
import numpy as np

sys.path.insert(0, "/opt/trn_rl_repo")

import concourse.bass as bass  # noqa: E402
import concourse.tile as tile  # noqa: E402
from concourse import bacc, bass_isa, mybir  # noqa: E402
from concourse.bass import ds, ts  # noqa: E402
from concourse.bass_utils import run_bass_kernel_spmd  # noqa: E402

F32 = mybir.dt.float32
BF16 = mybir.dt.bfloat16
NPBF16 = ml_dtypes.bfloat16

B, CL, CI, CO = 2, 256, 512, 128
H = W = 128
P = H * W                    # 16384 pixels per batch
CHUNK = 2048                 # attention chunk (pixels)
NCH = P // CHUNK             # 8 chunks per batch
NCORES = 8
UPC = (B * NCH) // NCORES    # units (b,chunk) per core = 2
EPS = 1e-5
QB = 1024                    # q-block width (one 2-bank PSUM matmul)
NQB = CHUNK // QB            # 2
KSL = CHUNK // 128           # 16 k-pixel slices per chunk
NSL = NQB * KSL              # 32 (qb, slice) score tiles per unit
NCI_IMG = CI // 128          # 4 contraction slices for image proj
NCI_LID = CL // 128          # 2 for lidar proj

_PROGRAM = None              # compiled Bass program, built once per process
LAST_RESULTS = None          # BassKernelResults of the last kernel() call


def _build_program():
    nc = bacc.Bacc("TRN2", target_bir_lowering=False, debug=False,
                   num_devices=NCORES)

    # Per-core DRAM inputs (pre-sharded, bf16 on host, constants packed so
    # the startup path is few DMA issues).
    ximg = nc.dram_tensor("ximg", [UPC, NCI_IMG, 128, CHUNK], BF16,
                          kind="ExternalInput").ap()
    xlid = nc.dram_tensor("xlid", [UPC, NCI_LID, 128, CHUNK], BF16,
                          kind="ExternalInput").ap()
    wimg = nc.dram_tensor("wimg", [128, NCI_IMG * CO], BF16,
                          kind="ExternalInput").ap()
    wlid = nc.dram_tensor("wlid", [128, NCI_LID * CO], BF16,
                          kind="ExternalInput").ap()
    # columns: img_bias, lid_bias, exp_scale, pad
    scal = nc.dram_tensor("scal", [128, 4], F32, kind="ExternalInput").ap()
    # columns 0:128 identity, 128:256 ones/w1
    idon = nc.dram_tensor("idon", [128, 256], BF16, kind="ExternalInput").ap()
    y = nc.dram_tensor("y", [UPC, NQB, CO, QB], BF16, kind="ExternalOutput").ap()

    with tile.TileContext(nc) as tc, ExitStack() as ctx:
        const = ctx.enter_context(tc.tile_pool(name="const", bufs=1))
        xi_pool = ctx.enter_context(tc.tile_pool(name="xi", bufs=UPC * NCI_IMG))
        xl_pool = ctx.enter_context(tc.tile_pool(name="xl", bufs=UPC * NCI_LID))
        proj_pool = ctx.enter_context(tc.tile_pool(name="proj", bufs=4))
        kp_pool = ctx.enter_context(tc.tile_pool(name="kp", bufs=4))
        et_pool = ctx.enter_context(tc.tile_pool(name="et", bufs=8))
        tree_pool = ctx.enter_context(tc.tile_pool(name="tree", bufs=10))
        misc_pool = ctx.enter_context(tc.tile_pool(name="misc", bufs=2))
        res_pool = ctx.enter_context(tc.tile_pool(name="res", bufs=2))
        # PSUM (8 banks of [128, 2KB]):
        #   mm 2x[128,1024]f32 (4 banks) - scores, double buffered; also
        #     borrowed for proj and transposes; the lb matmul reuses the
        #     slice-15 score tile so the rotation parity is preserved
        #   av 2x[128,1024]f32 (4 banks) - AV accumulation, double buffered
        mm_psum = ctx.enter_context(tc.tile_pool(name="mmps", bufs=2, space="PSUM"))
        av_psum = ctx.enter_context(tc.tile_pool(name="avps", bufs=2, space="PSUM"))

        # ---- startup-critical DMAs on the sync queue, in dependency order
        # for the first projection group (lidar first) ----
        # DMA issues appear to be synchronous on their issuing queue, so
        # spread the startup-critical loads over three otherwise-idle queues:
        #   sync:   lidar h0 + lidar weights + scalars  (first proj group)
        #   vector: lidar h1 + image h1                 (second proj groups)
        #   gpsimd: image weights/ident + image h0, then unit-1 inputs
        xl = {0: [xl_pool.tile([128, CHUNK], BF16, name=f"xl_0_{ci}", tag="xl")
                  for ci in range(NCI_LID)]}
        xi = {0: [xi_pool.tile([128, CHUNK], BF16, name=f"xi_0_{ci}", tag="xi")
                  for ci in range(NCI_IMG)]}
        for ci in range(NCI_LID):
            nc.sync.dma_start(xl[0][ci][:, ts(0, QB)], xlid[0, ci, :, ts(0, QB)])
        wlid_t = const.tile([128, NCI_LID * CO], BF16)
        nc.sync.dma_start(wlid_t[:], wlid)
        scal_t = const.tile([128, 4], F32)
        nc.sync.dma_start(scal_t[:], scal)
        img_b = scal_t[:, ds(0, 1)]
        lid_b = scal_t[:, ds(1, 1)]
        esc = scal_t[:, ds(2, 1)]
        w1s = scal_t[:, ds(3, 1)]
        for ci in range(NCI_LID):
            nc.scalar.dma_start(xl[0][ci][:, ts(1, QB)], xlid[0, ci, :, ts(1, QB)])
        wimg_t = const.tile([128, NCI_IMG * CO], BF16)
        nc.gpsimd.dma_start(wimg_t[:], wimg)
        idon_t = const.tile([128, 256], BF16)
        nc.gpsimd.dma_start(idon_t[:], idon)
        ident_t = idon_t[:, ds(0, 128)]
        ones_t = idon_t[:, ds(128, 128)]
        for ci in range(NCI_IMG):
            nc.gpsimd.dma_start(xi[0][ci][:, ts(0, QB)], ximg[0, ci, :, ts(0, QB)])
        for ci in range(NCI_IMG):
            nc.gpsimd.dma_start(xi[0][ci][:, ts(1, QB)], ximg[0, ci, :, ts(1, QB)])
        for u in range(1, UPC):
            xl[u] = [xl_pool.tile([128, CHUNK], BF16, name=f"xl_{u}_{ci}",
                                  tag="xl") for ci in range(NCI_LID)]
            xi[u] = [xi_pool.tile([128, CHUNK], BF16, name=f"xi_{u}_{ci}",
                                  tag="xi") for ci in range(NCI_IMG)]
            for ci in range(NCI_LID):
                nc.gpsimd.dma_start(xl[u][ci][:], xlid[u, ci])
            for ci in range(NCI_IMG):
                nc.gpsimd.dma_start(xi[u][ci][:], ximg[u, ci])

        qT = {}
        kT = {}
        kpw = {}

        def emit_proj_part1(u):
            """Lidar projection (both q-blocks), image q-block 0, and the K
            transposes.  Epilogues on the scalar engine, which is idle in
            this phase."""
            qT[u] = proj_pool.tile([128, CHUNK], BF16, name=f"qT_{u}", tag="qT")
            kT[u] = proj_pool.tile([128, CHUNK], BF16, name=f"kT_{u}", tag="kT")
            for qb in range(NQB):
                ps = mm_psum.tile([128, QB], F32, name=f"psl_{u}_{qb}", tag="ps")
                for ci in range(NCI_LID):
                    for h in range(QB // 512):
                        nc.tensor.matmul(ps[:, ts(h, 512)], wlid_t[:, ts(ci, CO)],
                                         xl[u][ci][:, ds(qb * QB + h * 512, 512)],
                                         start=(ci == 0), stop=(ci == NCI_LID - 1))
                nc.scalar.activation(kT[u][:, ts(qb, QB)], ps[:],
                                     mybir.ActivationFunctionType.Relu,
                                     bias=lid_b)
            ps = mm_psum.tile([128, QB], F32, name=f"psi_{u}_0", tag="ps")
            for ci in range(NCI_IMG):
                for h in range(QB // 512):
                    nc.tensor.matmul(ps[:, ts(h, 512)], wimg_t[:, ts(ci, CO)],
                                     xi[u][ci][:, ds(h * 512, 512)],
                                     start=(ci == 0), stop=(ci == NCI_IMG - 1))
            nc.scalar.activation(qT[u][:, ds(0, QB)], ps[:],
                                 mybir.ActivationFunctionType.Relu,
                                 bias=img_b)
            kpw[u] = []

        def emit_transpose_group(u, g):
            """One 8-slice K transpose group, PSUM borrowed from the (still
            idle) AV pool so the score-tile rotation is untouched; rides the
            early-stream PE slack before the AV matmuls begin."""
            pt = av_psum.tile([128, 8 * 128], BF16, name=f"pt_{u}_{g}",
                              tag="av")
            for k in range(8):
                nc.tensor.transpose(pt[:, ts(k, 128)],
                                    kT[u][:, ts(g * 8 + k, 128)], ident_t)
            kpt = kp_pool.tile([128, 8 * 128], BF16, name=f"kp_{u}_{g}",
                               tag="kp")
            nc.vector.tensor_copy(kpt[:], pt[:])
            kpw[u].append(kpt)

        def emit_proj_part2(u):
            """Image q-block 1, slotted into the attention stream (PSUM from
            the AV pool, relu on the DVE) so neither the score-tile rotation
            nor the scalar engine's exp stream is interrupted."""
            ps = av_psum.tile([128, QB], F32, name=f"psi_{u}_1", tag="av")
            for ci in range(NCI_IMG):
                for h in range(QB // 512):
                    nc.tensor.matmul(ps[:, ts(h, 512)], wimg_t[:, ts(ci, CO)],
                                     xi[u][ci][:, ds(QB + h * 512, 512)],
                                     start=(ci == 0), stop=(ci == NCI_IMG - 1))
            nc.vector.tensor_scalar(qT[u][:, ds(QB, QB)], ps[:], img_b, 0.0,
                                    op0=mybir.AluOpType.add,
                                    op1=mybir.AluOpType.max)

        emit_proj_part1(0)
        for u in range(UPC):
            # ---- attention: one global slice pipeline across both q-blocks.
            # scores/exp run LOOKAHEAD slices ahead of the AV matmuls so the
            # in-order PE queue never waits on the scalar engine. ----
            LOOKAHEAD = 3
            ets = [None] * NSL
            pos = [None] * NQB
            ps_sc = [None] * NSL
            lvl1 = [None] * (KSL // 2)
            lvl2 = [None] * (KSL // 4)
            lvl3 = {}

            def qb_epilogue(qb, parts):
                """Denominator broadcast ((ones/w1)^T @ partial S tiles, a
                4-way PSUM accumulation so only one cheap DVE add separates
                the last exp from the lb matmul) into an AV-pool tile -- the
                score-tile rotation never sees it and its short consumer
                chain barely delays the next AV allocation -- then
                reciprocal + blend + store in 512-wide halves so the chain
                pipelines on the DVE."""
                pl = av_psum.tile([128, QB], F32, name=f"pl_{u}_{qb}",
                                  tag="av")
                for h in range(QB // 512):
                    for pi, part in enumerate(parts):
                        nc.tensor.matmul(pl[:, ts(h, 512)], ones_t,
                                         part[:, ts(h, 512)],
                                         start=(pi == 0),
                                         stop=(pi == len(parts) - 1))
                linv = misc_pool.tile([128, QB], F32, name=f"linv_{u}_{qb}",
                                      tag="linv")
                tmp = misc_pool.tile([128, QB], F32, name=f"tmp_{u}_{qb}",
                                     tag="tmp")
                res = res_pool.tile([128, QB], BF16, name=f"res_{u}_{qb}",
                                    tag="res")
                for h in range(QB // 512):
                    nc.vector.reciprocal_approx_fast(linv[:, ts(h, 512)],
                                                     pl[:, ts(h, 512)])
                    nc.vector.tensor_mul(tmp[:, ts(h, 512)],
                                         pos[qb][:, ts(h, 512)],
                                         linv[:, ts(h, 512)])
                    nc.vector.tensor_add(res[:, ts(h, 512)], tmp[:, ts(h, 512)],
                                         qT[u][:, ds(qb * QB + h * 512, 512)])
                    nc.sync.dma_start(y[u, qb, :, ts(h, 512)],
                                      res[:, ts(h, 512)])

            for g in range(NSL + LOOKAHEAD):
                if g < NSL:
                    qb, i = divmod(g, KSL)
                    ps = mm_psum.tile([128, QB], F32,
                                      name=f"pss_{u}_{qb}_{i}", tag="ps")
                    ps_sc[g] = ps
                    for h in range(QB // 512):
                        nc.tensor.matmul(ps[:, ts(h, 512)], kT[u][:, ts(i, 128)],
                                         qT[u][:, ds(qb * QB + h * 512, 512)],
                                         start=True, stop=True)
                    et = et_pool.tile([128, QB], BF16,
                                      name=f"et_{u}_{qb}_{i}", tag="et")
                    nc.scalar.activation(et[:], ps[:],
                                         mybir.ActivationFunctionType.Exp,
                                         scale=esc)
                    ets[g] = et
                    # binary-tree accumulation of S (all-bf16 2x DVE adds)
                    if i % 2 == 1:
                        t = tree_pool.tile([128, QB], BF16,
                                           name=f"t1_{u}_{qb}_{i}", tag="t1")
                        nc.vector.tensor_add(t[:], ets[g - 1][:], et[:])
                        lvl1[i // 2] = t
                        if i % 4 == 3 and i < 12:
                            t2 = tree_pool.tile([128, QB], BF16,
                                                name=f"t2_{u}_{qb}_{i}",
                                                tag="t1")
                            nc.vector.tensor_add(t2[:], lvl1[i // 2 - 1][:],
                                                 t[:])
                            lvl2[i // 4] = t2
                            if i == 7:
                                t3 = tree_pool.tile([128, QB], BF16,
                                                    name=f"t3_{u}_{qb}_{i}",
                                                    tag="t1")
                                nc.vector.tensor_add(t3[:],
                                                     lvl2[0][:], t2[:])
                                lvl3[(qb, 0)] = t3
                    if 1 <= g <= 2:
                        emit_transpose_group(u, g - 1)
                    elif g == 3:
                        emit_proj_part2(u)
                elif g == NSL and u + 1 < UPC:
                    # overlap the next unit's projections with this unit's
                    # AV drain + epilogue (scalar engine idles here anyway)
                    emit_proj_part1(u + 1)
                j = g - LOOKAHEAD
                if j >= 0:
                    qbj, i = divmod(j, KSL)
                    if i == 0:
                        pos[qbj] = av_psum.tile([128, QB], F32,
                                                name=f"po_{u}_{qbj}", tag="av")
                    kslice = kpw[u][i // 8][:, ts(i % 8, 128)]
                    for h in range(QB // 512):
                        nc.tensor.matmul(pos[qbj][:, ts(h, 512)], kslice,
                                         ets[j][:, ts(h, 512)],
                                         start=(i == 0), stop=(i == KSL - 1))
                    if i == KSL - 1:
                        qb_epilogue(qbj,
                                    [lvl3[(qbj, 0)], lvl2[2], lvl1[6],
                                     lvl1[7]])

    nc.compile()
    return nc


def _shard_inputs(inputs):
    """Build the 8 per-core input maps from the full input dict."""
    mw = np.asarray(inputs["modality_weights"], np.float64)
    e = np.exp(mw - mw.max())
    w = (e / e.sum()).astype(np.float64)
    w0, w1 = float(w[0]), float(w[1])

    def bn_fold(gamma, beta, mean, var, mul):
        g = np.asarray(gamma, np.float64)
        b = np.asarray(beta, np.float64)
        m = np.asarray(mean, np.float64)
        v = np.asarray(var, np.float64)
        scale = g / np.sqrt(v + EPS) * mul
        bias = (b - m * g / np.sqrt(v + EPS)) * mul
        return scale, bias.astype(np.float32)

    i_s, i_b = bn_fold(inputs["image_gamma"], inputs["image_beta"],
                       inputs["image_mean"], inputs["image_var"], w0)
    l_s, l_b = bn_fold(inputs["lidar_gamma"], inputs["lidar_beta"],
                       inputs["lidar_mean"], inputs["lidar_var"], 1.0)

    # weight slices with the BN scale folded in, packed [cin(128), ci*CO]
    wi = (np.asarray(inputs["image_w"], np.float64) * i_s[:, None]).T.reshape(
        NCI_IMG, 128, CO).transpose(1, 0, 2).reshape(128, NCI_IMG * CO)
    wi = np.ascontiguousarray(wi).astype(NPBF16)
    wl = (np.asarray(inputs["lidar_w"], np.float64) * l_s[:, None]).T.reshape(
        NCI_LID, 128, CO).transpose(1, 0, 2).reshape(128, NCI_LID * CO)
    wl = np.ascontiguousarray(wl).astype(NPBF16)

    scal = np.zeros((128, 4), np.float32)
    scal[:, 0] = i_b
    scal[:, 1] = l_b
    scal[:, 2] = 1.0 / (w0 * math.sqrt(CO))
    scal[:, 3] = w1
    idon = np.zeros((128, 256), np.float32)
    idon[:, :128] = np.eye(128, dtype=np.float32)
    # ones carry 1/w1 so linv = 1/pl = w1/denominator
    idon[:, 128:] = 1.0 / w1
    idon = idon.astype(NPBF16)

    # full features reshaped to (B, nchunks, C, 2048), bf16
    img = np.asarray(inputs["image_features"], np.float32).reshape(
        B, CI, NCH, CHUNK).astype(NPBF16)
    lid = np.asarray(inputs["lidar_features"], np.float32).reshape(
        B, CL, NCH, CHUNK).astype(NPBF16)

    in_maps = []
    for core in range(NCORES):
        ximg = np.empty((UPC, NCI_IMG, 128, CHUNK), NPBF16)
        xlid = np.empty((UPC, NCI_LID, 128, CHUNK), NPBF16)
        for ul in range(UPC):
            un = core * UPC + ul
            b, c = un // NCH, un % NCH
            ximg[ul] = img[b, :, c, :].reshape(NCI_IMG, 128, CHUNK)
            xlid[ul] = lid[b, :, c, :].reshape(NCI_LID, 128, CHUNK)
        in_maps.append({
            "ximg": ximg, "xlid": xlid, "wimg": wi, "wlid": wl,
            "scal": scal, "idon": idon,
        })
    return in_maps


def kernel(**inputs) -> np.ndarray:
    global _PROGRAM, LAST_RESULTS
    if _PROGRAM is None:
        _PROGRAM = _build_program()
    nc = _PROGRAM

    in_maps = _shard_inputs(inputs)
    trace = os.environ.get("BASS_KERNEL_TRACE", "0") == "1"
    tmpdir = os.environ.get("BASS_KERNEL_TRACE_DIR") or None
    if tmpdir:
        os.makedirs(tmpdir, exist_ok=True)
    results = run_bass_kernel_spmd(nc, in_maps, core_ids=list(range(NCORES)),
                                   trace=trace, tmpdir=tmpdir)
    LAST_RESULTS = results

    out = np.empty((B, CO, H, W), np.float32)
    outv = out.reshape(B, CO, NCH, NQB, QB)
    for core in range(NCORES):
        yc = np.asarray(results.results[core]["y"], dtype=np.float32)
        for ul in range(UPC):
            un = core * UPC + ul
            b, c = un // NCH, un % NCH
            outv[b, :, c, :, :] = yc[ul].transpose(1, 0, 2)
    return out


if __name__ == "__main__":
    rng = np.random.default_rng(0)
    inputs = {
        "lidar_features": rng.standard_normal((B, CL, H, W), np.float32),
        "image_features": rng.standard_normal((B, CI, H, W), np.float32),
        "lidar_w": rng.standard_normal((CO, CL), np.float32) * np.sqrt(2.0 / CO),
        "lidar_gamma": np.ones(CO, np.float32),
        "lidar_beta": np.zeros(CO, np.float32),
        "lidar_mean": rng.standard_normal(CO).astype(np.float32) * 0.1,
        "lidar_var": rng.uniform(0.5, 1.5, CO).astype(np.float32),
        "image_w": rng.standard_normal((CO, CI), np.float32) * np.sqrt(2.0 / CO),
        "image_gamma": np.ones(CO, np.float32),
        "image_beta": np.zeros(CO, np.float32),
        "image_mean": rng.standard_normal(CO).astype(np.float32) * 0.1,
        "image_var": rng.uniform(0.5, 1.5, CO).astype(np.float32),
        "modality_weights": np.ones(2, np.float32),
    }
    out = kernel(**inputs)
    print("kernel out:", out.shape, out.dtype, float(np.abs(out).mean()))
